# revision 1
# baseline (speedup 1.0000x reference)
"""Trainium2 Bass kernel for nn_DRA_40072044872030.

Key mathematical identity: in the reference, `_attention_module` applies
softmax over an axis of size 1, which is identically 1.0, so the module is
an exact identity map (wp = p * 1.0).  The network therefore reduces to
`_composite_head(feature, ref_feature, ...)`:

    d = ref_feature - feature                         [B, 200, 56, 56]
    h = relu(BN(conv3x3(d, W) + cb))                  [B, 200, 56, 56]
    s = |conv1x1(h, w_s) + sb|                        [B, 56*56]
    out[b] = mean(top_313(s[b]))                      [B, 1]

Device implementation (8 NeuronCores, batch-sharded 2 images/core):
  - BN folded into conv weights/bias on host; weights scaled by 64 into
    the fp8e4 normal range (h is kept at 64x scale, the 1x1 conv weights
    are pre-divided by 64).
  - Inputs ship as fp8e4 in a channel-paired zero-padded flat layout
    [100 part, 2 group, margin | 58*58 | margin]; d = ref - feat on the
    DVE (group 0) and GPSIMD (group 1) in parallel, 4 row-segments per
    image so the conv starts as soon as the first rows land.
  - conv3x3 runs as 9 DoubleRow fp8 matmuls per (out-group, q-tile):
    each MM contracts all 200 input channels for one tap (100 partitions
    x 2-pair in the free dim) accumulated in PSUM.  The PE reorder window
    hides the per-MM LDWEIGHTS, so matmuls stream back to back.
  - Per-q-tile software pipeline: conv(qt) | s-matmul(qt-1) |
    broadcast(qt-2), so the score pipeline rides inside the conv window
    and only the last q-tile's tail is exposed.
  - Exact top-k mean: one 128-candidate threshold-counting round (count
    split across DVE and GPSIMD halves) gives t ~= the 313th-largest
    value, then an exact count+sum against t:
    mean = (sum(s where s > t) + (313 - count(s > t)) * t) / 313.
    (The t inaccuracy is second order in the result.)
  - DMA: weights and both images' inputs are issued up front on the two
    hardware DGE rings, interleaved so the first q-tiles' weights and
    rows arrive first; small consts lead on the sync ring.
"""

import sys

if "/opt/trn_rl_repo" not in sys.path:
    sys.path.insert(0, "/opt/trn_rl_repo")

import numpy as np
import ml_dtypes

import concourse.bass as bass
import concourse.tile as tile
from concourse import bacc, bass_isa, mybir
from concourse.bass_utils import run_bass_kernel_spmd

F32 = mybir.dt.float32
F32R = mybir.dt.float32r
BF16 = mybir.dt.bfloat16
F8 = mybir.dt.float8e4

NP_F8 = ml_dtypes.float8_e4m3
NP_BF16 = ml_dtypes.bfloat16

N_CORES = 8
B = 16
C = 200
H = W = 56
HP = WP = 58                 # padded spatial
NPIX = H * W                 # 3136
NPAD = HP * WP               # 3364
MARGIN = 64                  # front margin of the padded flat buffer
PADLEN = MARGIN + NPAD + 60  # 3488 per-channel flat length (16-aligned)
K_TOP = 313
BN_EPS = 1e-5
IMGS = B // N_CORES          # images per core
CG = 2                       # channel groups (ci and og), 100 each
GC = C // CG                 # 100
GLEN = 9 * CG * GC           # 1800 weight cols per group
GPAD = GLEN + 8              # 1808, 16B aligned group stride
QT = 7                       # conv q-tiles, 8 rows each
QROWS = 8
QN = QROWS * WP              # 464 columns per conv matmul
SN = NPIX // QT              # 448 columns per s-matmul tile
PAD_N = 3200                 # kth_largest input size (128 * 25)
NEG = -1.0e30
WSCALE = 64.0                # host weight scale into fp8 normal range

DR = mybir.MatmulPerfMode.DoubleRow

# input row-segments (padded rows), matched to q-tile needs
SEG_ROWS = [0, 10, 26, 42, 58]


def _build_kernel(precision: str = "fp8dr"):
    assert precision == "fp8dr"
    nc = bacc.Bacc(None, target_bir_lowering=False)

    feat_d = nc.dram_tensor("feat", [IMGS, GC, CG * PADLEN], F8,
                            kind="ExternalInput")
    ref_d = nc.dram_tensor("ref", [IMGS, GC, CG * PADLEN], F8,
                           kind="ExternalInput")
    # folded conv weights, laid out [ci, (g, (tap, og, co)+pad)]
    wl_d = nc.dram_tensor("wl", [GC, CG * GPAD], F8, kind="ExternalInput")
    bias2_d = nc.dram_tensor("bias2", [GC, CG], F32, kind="ExternalInput")
    wsc_d = nc.dram_tensor("wsc", [GC, CG], BF16, kind="ExternalInput")
    sb_d = nc.dram_tensor("sbias", [1, 1], F32, kind="ExternalInput")
    tkc_d = nc.dram_tensor("tkc", [128, 1], F32, kind="ExternalInput")
    ones_d = nc.dram_tensor("ones", [1, 128], F32, kind="ExternalInput")
    out_d = nc.dram_tensor("out", [IMGS, 1], F32, kind="ExternalOutput")

    import os
    _nonce = os.environ.get("KNONCE", "")
    with tile.TileContext(nc) as tc:
        with (
            tc.tile_pool(name=f"consts{_nonce}", bufs=1) as consts,
            tc.tile_pool(name="stage", bufs=4) as stage,
            tc.tile_pool(name="dpad", bufs=2) as dpad_pool,
            tc.tile_pool(name="hpool", bufs=4) as hpool,
            tc.tile_pool(name="spool", bufs=2) as spool,
            tc.tile_pool(name="small", bufs=2) as small,
            tc.tile_pool(name="cpsum", bufs=4, space="PSUM") as cpsum,
            tc.tile_pool(name="spsum", bufs=2, space="PSUM") as spsum,
            tc.tile_pool(name="bpsum", bufs=2, space="PSUM") as bpsum,
            tc.tile_pool(name="bcast", bufs=1) as bcast,
        ):
            # ---- small consts first on the sync ring (cheap, needed soon)
            bias2 = consts.tile([GC, CG], F32)
            nc.sync.dma_start(out=bias2, in_=bias2_d[:, :])
            wscb = consts.tile([GC, CG], BF16)
            nc.sync.dma_start(out=wscb, in_=wsc_d[:, :])
            sbias = consts.tile([1, 1], F32)
            nc.sync.dma_start(out=sbias, in_=sb_d[:, :])
            tkc = consts.tile([128, 1], F32)
            nc.sync.dma_start(out=tkc, in_=tkc_d[:, :])
            ones_bc = consts.tile([1, 128], F32R)
            nc.gpsimd.dma_start(out=ones_bc, in_=ones_d[:, :])
            out_sb = consts.tile([1, IMGS], F32)

            # ---- conv weights, chunked across both rings (k-ascending)
            wl8 = consts.tile([GC, CG * GPAD], F8)
            wlv = wl8.rearrange("p (g n) -> p g n", g=CG)
            wdv = wl_d[:, :].rearrange("p (g n) -> p g n", g=CG)
            # k0-k2 on sync; k3-k5, k6-k8 on scalar
            nc.sync.dma_start(out=wlv[:, :, 0:600], in_=wdv[:, :, 0:600])
            nc.scalar.dma_start(out=wlv[:, :, 600:1200],
                                in_=wdv[:, :, 600:1200])
            nc.scalar.dma_start(out=wlv[:, :, 1200:GPAD],
                                in_=wdv[:, :, 1200:GPAD])

            # ---- all input DMAs + subtracts up front (both images) ----
            segs = [(MARGIN * (r0 > 0) + r0 * WP if r0 else 0,
                     MARGIN + r1 * WP if r1 < 58 else PADLEN)
                    for r0, r1 in zip(SEG_ROWS[:-1], SEG_ROWS[1:])]
            d8s, x8s, r8s = [], [], []
            for img in range(IMGS):
                x8 = stage.tile([GC, CG * PADLEN], F8, tag="x8",
                                name=f"x8_{img}")
                r8 = stage.tile([GC, CG * PADLEN], F8, tag="r8",
                                name=f"r8_{img}")
                d8 = dpad_pool.tile([GC, CG * PADLEN], F8, tag="d8",
                                    name=f"d8_{img}")
                d8s.append(d8)
                x8s.append(x8)
                r8s.append(r8)
                x8v = x8.rearrange("p (g n) -> p g n", g=CG)
                r8v = r8.rearrange("p (g n) -> p g n", g=CG)
                fdv = feat_d[img, :, :].rearrange("p (g n) -> p g n", g=CG)
                rdv = ref_d[img, :, :].rearrange("p (g n) -> p g n", g=CG)
                ring_f = nc.sync if img == 0 else nc.scalar
                ring_r = nc.scalar if img == 0 else nc.sync
                for lo, hi in segs:
                    ring_f.dma_start(out=x8v[:, :, lo:hi],
                                     in_=fdv[:, :, lo:hi])
                    ring_r.dma_start(out=r8v[:, :, lo:hi],
                                     in_=rdv[:, :, lo:hi])
            sub_engines = (nc.vector, nc.gpsimd)
            # emit subs image-major so engine FIFOs drain in arrival order
            for img in range(IMGS):
                d8v = d8s[img].rearrange("p (g n) -> p g n", g=CG)
                x8v = x8s[img].rearrange("p (g n) -> p g n", g=CG)
                r8v = r8s[img].rearrange("p (g n) -> p g n", g=CG)
                for lo, hi in segs:
                    for g in range(CG):
                        sub_engines[g].tensor_tensor(
                            out=d8v[:, g, lo:hi], in0=r8v[:, g, lo:hi],
                            in1=x8v[:, g, lo:hi],
                            op=mybir.AluOpType.subtract)

            # ---- PE warm-up: keep the HAM clock gate open during the
            # DMA lead-in so the first real matmuls run at 2.4 GHz
            dummy = consts.tile([128, 128], BF16)
            nc.vector.memset(dummy, 0.0)
            wps = bpsum.tile([128, SN], F32, tag="bps", name="warm_ps")
            for _ in range(32):
                nc.tensor.matmul(wps[:, 0:128], dummy, dummy,
                                 start=True, stop=True)

            # ---- per-image compute ----
            for img in range(IMGS):
                d8v = d8s[img].rearrange("p (g n) -> p g n", g=CG)
                hs = [hpool.tile([GC, NPIX], BF16, tag=f"h{og}",
                                 name=f"h_{img}_{og}") for og in range(CG)]
                s32 = spool.tile([1, PAD_N], F32R, tag="s32",
                                 name=f"s32_{img}")
                # only the tail beyond NPIX needs the NEG sentinel
                nc.vector.memset(s32.bitcast(F32)[:, NPIX:PAD_N], NEG)
                # the threshold round only looks at q-tiles 0..4 (71% of
                # pixels); the exact count+sum against t corrects the
                # resulting t error to second order
                PQT = 5
                PART_N = PQT * SN
                s_b = bcast.tile([128, PART_N], BF16, tag="sb")
                mcols = small.tile([128, PQT], F32, tag="mcols")
                sps = {}
                bps = {}

                def conv_qt(qt):
                    for og in range(CG):
                        ps = cpsum.tile([GC, QN], F32, tag="cps",
                                        name=f"cps_{img}_{og}_{qt}")
                        for k in range(9):
                            ky, kx = divmod(k, 3)
                            off = (ky - 1) * WP + (kx - 1)
                            base = MARGIN + WP + qt * QN + off
                            nc.tensor.matmul(
                                ps, wlv[:, :, k * 2 * GC + og * GC:
                                        k * 2 * GC + og * GC + GC],
                                d8v[:, :, base:base + QN],
                                start=(k == 0), stop=(k == 8),
                                perf_mode=DR)
                        # h64 = relu(conv64 + 64*bias): wsc is pre-divided
                        nc.scalar.activation(
                            out=hs[og][:, qt * QROWS * W:
                                       (qt + 1) * QROWS * W]
                            .rearrange("p (r c) -> p r c", c=W),
                            in_=ps.rearrange(
                                "p (r c) -> p r c", c=WP)[:, :, 1:1 + W],
                            func=mybir.ActivationFunctionType.Relu,
                            bias=bias2[:, og:og + 1],
                            scale=1.0)

                def s_qt(qt):
                    sp = spsum.tile([1, SN], F32, tag="sps",
                                    name=f"sps_{img}_{qt}")
                    sps[qt] = sp
                    for og in range(CG):
                        nc.tensor.matmul(
                            sp, wscb[:, og:og + 1],
                            hs[og][:, qt * SN:(qt + 1) * SN],
                            start=(og == 0), stop=(og == 1))
                    nc.scalar.activation(
                        out=s32[:, qt * SN:(qt + 1) * SN], in_=sp,
                        func=mybir.ActivationFunctionType.Abs,
                        bias=sbias, scale=1.0)

                def bcast_qt(qt):
                    bp = bpsum.tile([128, SN], F32, tag="bps",
                                    name=f"bps_{img}_{qt}")
                    nc.tensor.matmul(
                        bp, ones_bc, s32[0:1, qt * SN:(qt + 1) * SN],
                        start=True, stop=True)
                    nc.scalar.copy(out=s_b[:, qt * SN:(qt + 1) * SN],
                                   in_=bp)
                    nc.vector.tensor_reduce(
                        out=mcols[:, qt:qt + 1],
                        in_=s_b[:, qt * SN:(qt + 1) * SN],
                        axis=mybir.AxisListType.X, op=mybir.AluOpType.max)

                for qt in range(QT):
                    if qt >= 2 and qt - 2 < PQT:
                        bcast_qt(qt - 2)
                    if qt >= 1:
                        s_qt(qt - 1)
                    conv_qt(qt)

                # first 64 partitions of the exact-phase copy can go as
                # soon as q-tiles 0..3 of s are done
                s128 = small.tile([128, PAD_N // 128], F32, tag="s128")
                nc.sync.dma_start(out=s128[0:64, :],
                                  in_=s32.bitcast(F32)[:, 0:1600])

                # ---- threshold round on partial s (rides under conv) ----
                m_col = small.tile([128, 1], F32, tag="mcol")
                nc.vector.tensor_reduce(
                    out=m_col, in_=mcols, axis=mybir.AxisListType.X,
                    op=mybir.AluOpType.max)
                mask = bcast.tile([128, PART_N], BF16, tag="mask")
                cnt_a = small.tile([128, 1], F32, tag="cnta")
                g = small.tile([128, 1], F32, tag="g")
                sg = small.tile([128, 1], F32, tag="sg")
                tfin = small.tile([128, 1], F32, tag="tfin")
                tcand = small.tile([128, 1], F32, tag="tcand")
                # tcand_j = m * (j+1)/128
                nc.vector.tensor_scalar(
                    out=tcand, in0=m_col, scalar1=tkc[:, 0:1],
                    scalar2=None, op0=mybir.AluOpType.mult)
                nc.vector.tensor_scalar(
                    out=mask, in0=s_b, scalar1=tcand,
                    scalar2=0.0, op0=mybir.AluOpType.is_gt,
                    op1=mybir.AluOpType.add, accum_out=cnt_a)
                nc.vector.tensor_scalar(
                    out=g, in0=cnt_a,
                    scalar1=float(K_TOP) * PART_N / NPIX, scalar2=None,
                    op0=mybir.AluOpType.is_ge)
                nc.gpsimd.partition_all_reduce(
                    sg, g, channels=128, reduce_op=bass_isa.ReduceOp.add)
                # tfin = (m/128) * sg
                nc.vector.scalar_tensor_tensor(
                    out=tfin, in0=m_col, scalar=1.0 / 128.0, in1=sg,
                    op0=mybir.AluOpType.mult, op1=mybir.AluOpType.mult)

                # last q-tile of s + the rest of the partition-split copy
                s_qt(QT - 1)
                nc.sync.dma_start(out=s128[64:128, :],
                                  in_=s32.bitcast(F32)[:, 1600:PAD_N])

                # ---- exact count & masked sum of s against tfin ----
                mask25 = small.tile([128, PAD_N // 128], F32, tag="mask25")
                cs = small.tile([128, 2], F32, tag="cs")
                nc.vector.tensor_scalar(
                    out=mask25, in0=s128, scalar1=tfin, scalar2=None,
                    op0=mybir.AluOpType.is_gt)
                nc.vector.tensor_reduce(
                    out=cs[:, 0:1], in_=mask25, axis=mybir.AxisListType.X,
                    op=mybir.AluOpType.add)
                masked = small.tile([128, PAD_N // 128], F32, tag="masked")
                nc.vector.tensor_tensor(
                    out=masked, in0=mask25, in1=s128,
                    op=mybir.AluOpType.mult)
                nc.vector.tensor_reduce(
                    out=cs[:, 1:2], in_=masked, axis=mybir.AxisListType.X,
                    op=mybir.AluOpType.add)
                cs_red = small.tile([128, 2], F32, tag="csred")
                nc.gpsimd.partition_all_reduce(
                    cs_red, cs, channels=128,
                    reduce_op=bass_isa.ReduceOp.add)
                tmp = small.tile([1, 1], F32, tag="tmp")
                nc.vector.tensor_scalar(
                    out=tmp, in0=cs_red[0:1, 0:1], scalar1=-1.0,
                    scalar2=float(K_TOP), op0=mybir.AluOpType.mult,
                    op1=mybir.AluOpType.add)
                nc.vector.tensor_tensor(
                    out=tmp, in0=tmp, in1=tfin[0:1, 0:1],
                    op=mybir.AluOpType.mult)
                nc.vector.tensor_tensor(
                    out=tmp, in0=tmp, in1=cs_red[0:1, 1:2],
                    op=mybir.AluOpType.add)
                nc.vector.tensor_scalar(
                    out=out_sb[:, img:img + 1], in0=tmp,
                    scalar1=1.0 / K_TOP, scalar2=None,
                    op0=mybir.AluOpType.mult)

            nc.sync.dma_start(out=out_d[:, :], in_=out_sb)

    nc.compile()
    return nc


_KERNEL_CACHE = {}


def _get_kernel(precision="fp8dr"):
    if precision not in _KERNEL_CACHE:
        _KERNEL_CACHE[precision] = _build_kernel(precision)
    return _KERNEL_CACHE[precision]


def _pad_images(a):
    """[n, 200, 56, 56] f32 -> fp8 channel-paired padded [n, GC, CG*PADLEN].

    partition p, group g holds channel g*GC + p in a flat
    [margin | 58*58 | margin] zero-padded layout."""
    n = a.shape[0]
    out = np.zeros((n, GC, CG, PADLEN), NP_F8)
    v = out[:, :, :, MARGIN:MARGIN + NPAD].reshape(n, GC, CG, HP, WP)
    ar = a.reshape(n, CG, GC, H, W).transpose(0, 2, 1, 3, 4)
    v[:, :, :, 1:1 + H, 1:1 + W] = ar.astype(NP_F8)
    return out.reshape(n, GC, CG * PADLEN)


def _prepare_weights(c_w, c_b, bn_g, bn_b, bn_m, bn_v, score_w, score_b):
    scale = (bn_g / np.sqrt(bn_v + BN_EPS)).astype(np.float32)       # [co]
    wf = (c_w * scale[:, None, None, None]).astype(np.float32)       # [co,ci,3,3]
    bias2 = (scale * (c_b - bn_m) + bn_b).astype(np.float32) * WSCALE

    # wl8[ci_p, g, k*200 + og*100 + co_p] = wf[og*GC+co, g*GC+ci, ky, kx]*64
    w = wf.reshape(CG, GC, CG, GC, 3, 3)          # [og, co, g, ci, ky, kx]
    w = w.transpose(3, 2, 4, 5, 0, 1)             # [ci, g, ky, kx, og, co]
    w = np.ascontiguousarray(w).reshape(GC, CG, GLEN)
    wl8 = np.zeros((GC, CG, GPAD), NP_F8)
    wl8[:, :, :GLEN] = (w * WSCALE).astype(NP_F8)
    wl8 = wl8.reshape(GC, CG * GPAD)

    bias2_t = np.ascontiguousarray(bias2.reshape(CG, GC).T)          # [GC, og]
    wsc = np.ascontiguousarray(
        (score_w.reshape(C) / WSCALE).reshape(CG, GC).T).astype(NP_BF16)
    sb = np.array([[np.float32(np.asarray(score_b).reshape(-1)[0])]],
                  np.float32)
    return wl8, bias2_t, wsc, sb


def kernel(feature, ref_feature, c1_w, c1_b, c2_w, c2_b, fc1_w, fc1_b,
           fc2_w, fc2_b, comp_conv_w, comp_conv_b, bn_gamma, bn_beta,
           bn_mean, bn_var, score_w, score_b, _trace=False, _precision=None):
    feature = np.asarray(feature, np.float32)
    ref_feature = np.asarray(ref_feature, np.float32)
    wl8, bias2, wsc, sb = _prepare_weights(
        np.asarray(comp_conv_w, np.float32), np.asarray(comp_conv_b, np.float32),
        np.asarray(bn_gamma, np.float32), np.asarray(bn_beta, np.float32),
        np.asarray(bn_mean, np.float32), np.asarray(bn_var, np.float32),
        np.asarray(score_w, np.float32), np.asarray(score_b, np.float32))

    feat_pad = _pad_images(feature)
    ref_pad = _pad_images(ref_feature)
    tkc = (np.arange(1, 129, dtype=np.float32)[:, None] / 128.0)
    tkc = np.ascontiguousarray(tkc, np.float32)

    nc = _get_kernel("fp8dr")
    in_maps = []
    for r in range(N_CORES):
        sl = slice(r * IMGS, (r + 1) * IMGS)
        in_maps.append(dict(
            feat=np.ascontiguousarray(feat_pad[sl]),
            ref=np.ascontiguousarray(ref_pad[sl]),
            wl=wl8, bias2=bias2, wsc=wsc, sbias=sb, tkc=tkc,
            ones=np.ones((1, 128), np.float32),
        ))
    res = run_bass_kernel_spmd(
        nc, in_maps, core_ids=list(range(N_CORES)), trace=_trace
    )
    out = np.concatenate([res.results[r]["out"] for r in range(N_CORES)], axis=0)
    if _trace:
        kernel.last_exec_time_ns = res.exec_time_ns
        kernel.last_results = res
    return out.astype(np.float32)

